# revision 9
# baseline (speedup 1.0000x reference)
"""Trainium2 Bass kernel for an attention block (GroupNorm + single-head
self-attention + residual), B=8 x [64,64,64] channels-last, run data-parallel
across 8 NeuronCores (one batch per core).

Per-core math (S = H*W = 4096, C = 64):
  h  = (x - mu) * rsqrt(var + eps)          # GroupNorm(1 group), folded into W/b
  q  = h @ Wq.T + bq ; k = h @ Wk.T + bk ; v = h @ Wv.T + bv
  w  = softmax(q k^T / sqrt(C))             # no max-subtraction (scores ~ +-0.2)
  out = x + (w v) @ Wo.T + bo

Layouts: scores are computed transposed (sj on partitions, si on free) so the
exp'd tile feeds the A*V matmul directly as the moving operand; V carries an
appended ones column so the softmax denominator falls out of the same
accumulation. Division + residual happen after a PE transpose back to
[si, c] layout.
"""

import sys

for _p in ("/opt/trn_rl_repo",):
    if _p not in sys.path:
        sys.path.append(_p)

import numpy as np

import concourse.bass as bass
import concourse.bacc as bacc
import concourse.tile as tile
from concourse import mybir
from concourse.bass_utils import run_bass_kernel_spmd
from concourse.masks import make_identity

F32 = mybir.dt.float32
F32R = mybir.dt.float32r
BF16 = mybir.dt.bfloat16
AF = mybir.ActivationFunctionType
OP = mybir.AluOpType

B, H, W, C = 8, 64, 64, 64
S = H * W            # 4096
P = 128              # SBUF partitions
T = S // P           # 32 sj tiles
NB = S // 512        # 8 si blocks of 512
EPS = 1e-5

LAST_RESULTS = None
_CACHED_NC = None


def _r(ap):
    """View an fp32 AP as float32r (full-rate PE streaming for N>=256)."""
    return ap.bitcast(F32R)


def build_nc():
    nc = bacc.Bacc(trn_type="TRN2")

    x_e = nc.declare_dram_parameter("x", [S, C], F32, isOutput=False)
    w_e = {}
    b_e = {}
    for n in ("q", "k", "v", "o"):
        w_e[n] = nc.declare_dram_parameter(f"W{n}", [C, C], F32, isOutput=False)
        b_e[n] = nc.declare_dram_parameter(f"b{n}", [1, C], F32, isOutput=False)
    out_e = nc.declare_dram_parameter("out", [S, C], F32, isOutput=True)

    x_r = x_e.ap().rearrange("(t p) c -> p t c", p=P)        # [128, 32, 64]
    out_r = out_e.ap().rearrange("(nb q p) c -> nb p q c", q=4, p=P)

    with tile.TileContext(nc) as tc:
        with (
            tc.tile_pool(name="consts", bufs=1) as consts,
            tc.tile_pool(name="big", bufs=1) as big,
            tc.tile_pool(name="work", bufs=3) as work,
        ):
            # ---- persistent SBUF tensors ----
            x_sb = big.tile([P, T, C], F32)          # x, natural [si, c] tiles
            xT_sb = big.tile([C, S], F32R)            # x^T  [c, s]
            qT_sb = big.tile([C, S], BF16)            # q^T (rstd-scaled, biased)
            kT_sb = big.tile([C, S], BF16)
            v_sb = big.tile([P, T, C + 1], BF16)      # v tiles + ones column
            eT_sb = big.tile([P, T, 512], BF16)
            eT_sb2 = big.tile([P, T, 512], BF16)       # exp(scores^T) per si-block

            id128 = consts.tile([P, P], F32)
            make_identity(nc, id128)
            ones_col = consts.tile([P, 1], F32)
            nc.vector.memset(ones_col, 1.0)
            ones512_f = consts.tile([1, 512], F32)
            nc.vector.memset(ones512_f, 1.0)
            ones512 = consts.tile([1, 512], F32R)
            nc.vector.tensor_copy(ones512, ones512_f)
            ones32 = consts.tile([P, T], F32)
            nc.vector.memset(ones32, 1.0)

            w_sb = {}
            wT_sb = {}   # transposed (and rstd-scaled for q/k/v) weights [c_in, c_out]
            bias_sb = {}
            for n in ("q", "k", "v", "o"):
                w_sb[n] = consts.tile([C, C], F32, tag=f"w_{n}", name=f"w_{n}")
                nc.sync.dma_start(out=w_sb[n], in_=w_e[n][:, :])
                wT_sb[n] = consts.tile([C, C], F32R, tag=f"wT_{n}", name=f"wT_{n}")
                bias_sb[n] = consts.tile([1, C], F32R, tag=f"b_{n}", name=f"b_{n}")
            # bo lives on partition 64 so the K=1 bias matmul can pair with the
            # rowsum row of oc_sb (operands must share a base partition).
            bo_hi = consts.tile([C + 1, C], F32)
            nc.sync.dma_start(out=bo_hi[C : C + 1, :], in_=b_e["o"][:, :])

            nc.sync.dma_start(out=x_sb, in_=x_r)

            stats_sb = consts.tile([P, 3], F32)       # mean, var, mean^2 per partition
            moments = consts.tile([1, 4], F32)        # scratch for scalar math
            bvals = consts.tile([P, 2], F32)          # [mu, rstd] broadcast to all parts
            negmu = consts.tile([1, 1], F32)

            with tc.tile_pool(name="pre_ps", bufs=2, space="PSUM") as pps:
                # ---- GroupNorm stats: bn_stats per 512-chunk, then aggregate ----
                bnst = consts.tile([P, T * C // 512, 6], F32)
                x_flat = x_sb[:, :, :].rearrange("p t c -> p (t c)")
                for i in range(T * C // 512):
                    nc.vector.bn_stats(
                        out=bnst[:, i, :], in_=x_flat[:, bass.ts(i, 512)]
                    )
                nc.vector.bn_aggr(out=stats_sb[:, 0:2], in_=bnst)
                nc.vector.tensor_mul(
                    stats_sb[:, 2:3], stats_sb[:, 0:1], stats_sb[:, 0:1]
                )
                ssum_ps = pps.tile([1, 3], F32, tag="small")
                nc.tensor.matmul(ssum_ps, lhsT=ones_col, rhs=stats_sb)
                # moments: [E[mean_p], E[var_p], E[mean_p^2], _]
                nc.scalar.mul(moments[:, 0:3], ssum_ps, 1.0 / P)
                # var_total = E[var_p] + E[mean_p^2] - mu^2
                nc.vector.tensor_mul(moments[:, 3:4], moments[:, 0:1], moments[:, 0:1])
                nc.vector.tensor_sub(moments[:, 1:2], moments[:, 1:2], moments[:, 3:4])
                nc.vector.tensor_add(moments[:, 1:2], moments[:, 1:2], moments[:, 2:3])
                # rstd = exp(-0.5 * ln(var + eps)); Ln/Exp share one ACT table set
                eps_sb = consts.tile([1, 1], F32)
                nc.vector.memset(eps_sb, EPS)
                nc.scalar.activation(moments[:, 2:3], moments[:, 1:2], AF.Ln, bias=eps_sb)
                nc.scalar.activation(moments[:, 3:4], moments[:, 2:3], AF.Exp, scale=-0.5)
                nc.scalar.mul(negmu, moments[:, 0:1], -1.0)

                # broadcast [mu, rstd] to all 128 partitions via K=1 matmul
                pair = consts.tile([1, 2], F32)
                nc.vector.tensor_copy(pair[:, 0:1], moments[:, 0:1])
                nc.vector.tensor_copy(pair[:, 1:2], moments[:, 3:4])
                bc_ps = pps.tile([P, 2], F32, tag="small")
                nc.tensor.matmul(bc_ps, lhsT=ones512_f[0:1, 0:P], rhs=pair)
                nc.vector.tensor_copy(bvals, bc_ps)

                # ---- weights: transpose, scale q/k/v by rstd, fold mu into bias ----
                for n in ("q", "k", "v", "o"):
                    wt_ps = pps.tile([C, C], F32, tag="small")
                    nc.tensor.transpose(wt_ps, w_sb[n], id128[0:C, 0:C])
                    if n == "o":
                        nc.vector.tensor_copy(wT_sb[n], wt_ps)
                        continue
                    nc.scalar.mul(wT_sb[n], wt_ps, bvals[0:C, 1:2])
                    cs_ps = pps.tile([1, C], F32, tag="small")
                    nc.tensor.matmul(cs_ps, lhsT=ones_col[0:C, :], rhs=wT_sb[n].bitcast(F32))
                    b_tmp = consts.tile([1, C], F32, tag=f"braw_{n}", name=f"braw_{n}")
                    nc.sync.dma_start(out=b_tmp, in_=b_e[n][:, :])
                    # bias' = b + (-mu) * colsum(rstd * W^T)
                    nc.vector.scalar_tensor_tensor(
                        out=bias_sb[n],
                        in0=cs_ps,
                        scalar=negmu,
                        in1=b_tmp,
                        op0=OP.mult,
                        op1=OP.add,
                    )

                # ---- x^T via PE transpose, 4 tiles per PSUM bank ----
                for gq in range(T // 4):
                    tp_ps = pps.tile([C, 4 * P], F32, tag="tp")
                    for i in range(4):
                        t = gq * 4 + i
                        nc.tensor.transpose(
                            tp_ps[:, bass.ts(i, P)], x_sb[:, t, :], id128
                        )
                    nc.vector.tensor_copy(xT_sb[:, bass.ts(gq, 4 * P)], tp_ps)

                # ---- q^T, k^T ----
                for n, dst in (("q", qT_sb), ("k", kT_sb)):
                    for nb in range(NB):
                        qk_ps = pps.tile([C, 512], F32, tag="qk")
                        nc.tensor.matmul(
                            qk_ps,
                            lhsT=wT_sb[n],
                            rhs=xT_sb[:, bass.ts(nb, 512)],
                            start=True,
                            stop=False,
                        )
                        nc.tensor.matmul(
                            qk_ps,
                            lhsT=bias_sb[n],
                            rhs=ones512,
                            start=False,
                            stop=True,
                        )
                        nc.vector.tensor_copy(dst[:, bass.ts(nb, 512)], qk_ps)

                # ---- v tiles [sj, c] + bias, ones column ----
                for gv in range(T // 8):
                    v_ps = pps.tile([P, 8, C], F32, tag="vps")
                    for i in range(8):
                        t = gv * 8 + i
                        nc.tensor.matmul(
                            v_ps[:, i, :],
                            lhsT=xT_sb[:, bass.ts(t, P)],
                            rhs=wT_sb["v"],
                            start=True,
                            stop=False,
                        )
                        nc.tensor.matmul(
                            v_ps[:, i, :],
                            lhsT=ones512[0:1, 0:P],
                            rhs=bias_sb["v"],
                            start=False,
                            stop=True,
                        )
                    nc.vector.tensor_copy(
                        v_sb[:, bass.ts(gv, 8), 0:C], v_ps
                    )
                nc.vector.tensor_copy(v_sb[:, :, C], ones32)

            # ---- main attention loop over si blocks of 512, software-pipelined:
            # block nb's score-groups interleave with block nb-1's A*V chunks so
            # the PE never stalls on the exp (ScalarE) stage. eT is
            # double-buffered so exp(nb) doesn't wait on A*V(nb-1) reads.
            with (
                tc.tile_pool(name="sc_ps", bufs=1, space="PSUM") as sc_pool,
                tc.tile_pool(name="ot_ps", bufs=1, space="PSUM") as ot_pool,
                tc.tile_pool(name="z_ps", bufs=1, space="PSUM") as z_pool,
                tc.tile_pool(name="tr_ps", bufs=2, space="PSUM") as tr_pool,
            ):
                eT_bufs = [eT_sb, eT_sb2]

                def emit_scores_group(nb, g):
                    si = bass.ts(nb, 512)
                    sc_ps = sc_pool.tile([P, 4, 512], F32, tag="sc", name="sc_ps")
                    for i in range(4):
                        sj = g * 4 + i
                        nc.tensor.matmul(
                            sc_ps[:, i, :],
                            lhsT=kT_sb[:, bass.ts(sj, P)],
                            rhs=qT_sb[:, si],
                            start=True,
                            stop=True,
                        )
                    nc.scalar.activation(
                        out=eT_bufs[nb % 2][:, bass.ts(g, 4), :],
                        in_=sc_ps,
                        func=AF.Exp,
                        scale=float(C) ** -0.5,
                    )

                def emit_av_chunk(nb, g, ot_ps):
                    eT = eT_bufs[nb % 2]
                    for i in range(4):
                        sj = g * 4 + i
                        nc.tensor.matmul(
                            ot_ps,
                            lhsT=v_sb[:, sj, :],
                            rhs=eT[:, sj, :],
                            start=(sj == 0),
                            stop=(sj == T - 1),
                        )

                def emit_tail(nb, ot_ps):
                    oc_sb = work.tile([C + 1, 512], F32R, tag="oc", name="oc_sb")
                    nc.vector.tensor_copy(oc_sb, ot_ps)
                    # z^T = Wo @ o^T + bo x rowsum (divide happens post-transpose)
                    z_ps = z_pool.tile([C, 512], F32, tag="z", name="z_ps")
                    nc.tensor.matmul(
                        z_ps,
                        lhsT=wT_sb["o"],
                        rhs=oc_sb[0:C, :],
                        start=True,
                        stop=False,
                    )
                    nc.tensor.matmul(
                        z_ps,
                        lhsT=bo_hi[C : C + 1, :],
                        rhs=oc_sb[C : C + 1, :].bitcast(F32),
                        start=False,
                        stop=True,
                    )
                    zc_sb = work.tile([C + 1, 512], F32, tag="zc", name="zc_sb")
                    nc.vector.tensor_copy(zc_sb[0:C, :], z_ps)
                    nc.vector.tensor_copy(zc_sb[C : C + 1, :], oc_sb[C : C + 1, :])
                    # transpose back to [si, c], divide by rowsum, add residual
                    out_sb = work.tile([P, 4, C], F32, tag="outt", name="out_sb")
                    for q4 in range(4):
                        tr_ps = tr_pool.tile([P, C + 1], F32, tag="tr", name="tr_ps")
                        nc.tensor.transpose(
                            tr_ps, zc_sb[:, bass.ts(q4, P)], id128[0 : C + 1, 0 : C + 1]
                        )
                        rec_sb = work.tile([P, 1], F32, tag="rec", name="rec_sb")
                        nc.vector.reciprocal(rec_sb, tr_ps[:, C : C + 1])
                        nc.vector.scalar_tensor_tensor(
                            out=out_sb[:, q4, :],
                            in0=tr_ps[:, 0:C],
                            scalar=rec_sb,
                            in1=x_sb[:, nb * 4 + q4, :],
                            op0=OP.mult,
                            op1=OP.add,
                        )
                    nc.sync.dma_start(out=out_r[nb], in_=out_sb)

                ot_live = {}
                for nb in range(NB):
                    for g in range(T // 4):
                        emit_scores_group(nb, g)
                        if nb >= 1:
                            if g == 0:
                                ot_live[nb - 1] = ot_pool.tile(
                                    [C + 1, 512], F32, tag="ot", name="ot_ps"
                                )
                            emit_av_chunk(nb - 1, g, ot_live[nb - 1])
                    if nb >= 1:
                        emit_tail(nb - 1, ot_live.pop(nb - 1))
                ot_live[NB - 1] = ot_pool.tile([C + 1, 512], F32, tag="ot", name="ot_ps")
                for g in range(T // 4):
                    emit_av_chunk(NB - 1, g, ot_live[NB - 1])
                emit_tail(NB - 1, ot_live.pop(NB - 1))

    nc.finalize()
    return nc


def _get_nc():
    global _CACHED_NC
    if _CACHED_NC is None:
        _CACHED_NC = build_nc()
    return _CACHED_NC


def kernel(x, temb, Wq, bq, Wk, bk, Wv, bv, Wo, bo, **_unused):
    global LAST_RESULTS
    nc = _get_nc()
    x = np.ascontiguousarray(np.asarray(x, dtype=np.float32))
    shared = {
        "Wq": np.ascontiguousarray(Wq, dtype=np.float32),
        "Wk": np.ascontiguousarray(Wk, dtype=np.float32),
        "Wv": np.ascontiguousarray(Wv, dtype=np.float32),
        "Wo": np.ascontiguousarray(Wo, dtype=np.float32),
        "bq": np.asarray(bq, dtype=np.float32).reshape(1, C),
        "bk": np.asarray(bk, dtype=np.float32).reshape(1, C),
        "bv": np.asarray(bv, dtype=np.float32).reshape(1, C),
        "bo": np.asarray(bo, dtype=np.float32).reshape(1, C),
    }
    in_maps = [{"x": x[i].reshape(S, C), **shared} for i in range(B)]
    res = run_bass_kernel_spmd(nc, in_maps, core_ids=list(range(B)))
    LAST_RESULTS = res
    out = np.stack([res.results[i]["out"].reshape(H, W, C) for i in range(B)])
    return out.astype(np.float32)


# revision 11
# speedup vs baseline: 1.0187x; 1.0187x over previous
"""Trainium2 Bass kernel for an attention block (GroupNorm + single-head
self-attention + residual), B=8 x [64,64,64] channels-last, run data-parallel
across 8 NeuronCores (one batch per core).

Per-core math (S = H*W = 4096, C = 64):
  h  = (x - mu) * rsqrt(var + eps)          # GroupNorm(1 group), folded into W/b
  q  = h @ Wq.T + bq ; k = h @ Wk.T + bk ; v = h @ Wv.T + bv
  w  = softmax(q k^T / sqrt(C))             # no max-subtraction (scores ~ +-0.2)
  out = x + (w v) @ Wo.T + bo

Layouts: scores are computed transposed (sj on partitions, si on free) so the
exp'd tile feeds the A*V matmul directly as the moving operand; V carries an
appended ones column so the softmax denominator falls out of the same
accumulation. Division + residual happen after a PE transpose back to
[si, c] layout.
"""

import sys

for _p in ("/opt/trn_rl_repo",):
    if _p not in sys.path:
        sys.path.append(_p)

import numpy as np

import concourse.bass as bass
import concourse.bacc as bacc
import concourse.tile as tile
from concourse import mybir
from concourse.bass_utils import run_bass_kernel_spmd
from concourse.masks import make_identity

F32 = mybir.dt.float32
F32R = mybir.dt.float32r
BF16 = mybir.dt.bfloat16
FP8 = mybir.dt.float8e4
DR = mybir.MatmulPerfMode.DoubleRow
AF = mybir.ActivationFunctionType
OP = mybir.AluOpType

B, H, W, C = 8, 64, 64, 64
S = H * W            # 4096
P = 128              # SBUF partitions
T = S // P           # 32 sj tiles
NB = S // 512        # 8 si blocks of 512
EPS = 1e-5

LAST_RESULTS = None
_CACHED_NC = None


def _r(ap):
    """View an fp32 AP as float32r (full-rate PE streaming for N>=256)."""
    return ap.bitcast(F32R)


def build_nc():
    nc = bacc.Bacc(trn_type="TRN2")

    x_e = nc.declare_dram_parameter("x", [S, C], F32, isOutput=False)
    w_e = {}
    b_e = {}
    for n in ("q", "k", "v", "o"):
        w_e[n] = nc.declare_dram_parameter(f"W{n}", [C, C], F32, isOutput=False)
        b_e[n] = nc.declare_dram_parameter(f"b{n}", [1, C], F32, isOutput=False)
    out_e = nc.declare_dram_parameter("out", [S, C], F32, isOutput=True)

    x_r = x_e.ap().rearrange("(t p) c -> p t c", p=P)        # [128, 32, 64]
    out_r = out_e.ap().rearrange("(nb q p) c -> nb p q c", q=4, p=P)

    with tile.TileContext(nc) as tc:
        with (
            tc.tile_pool(name="consts", bufs=1) as consts,
            tc.tile_pool(name="big", bufs=1) as big,
            tc.tile_pool(name="work", bufs=3) as work,
        ):
            # ---- persistent SBUF tensors ----
            x_sb = big.tile([P, T, C], F32)          # x, natural [si, c] tiles
            xT_sb = big.tile([C, S], F32R)            # x^T  [c, s]
            qT_sb = big.tile([C // 2, 2, S], FP8)            # q^T (rstd-scaled, biased)
            kT_sb = big.tile([C // 2, 2, S], FP8)
            v_sb = big.tile([P, T, 80], FP8)      # v tiles + ones column
            eT_sb = big.tile([P, T, 512], FP8)
            eT_sb2 = big.tile([P, T, 512], FP8)       # exp(scores^T) per si-block

            id128 = consts.tile([P, P], F32)
            make_identity(nc, id128)
            ones_col = consts.tile([P, 1], F32)
            nc.vector.memset(ones_col, 1.0)
            ones512_f = consts.tile([1, 512], F32)
            nc.vector.memset(ones512_f, 1.0)
            ones512 = consts.tile([1, 512], F32R)
            nc.vector.tensor_copy(ones512, ones512_f)
            ones32 = consts.tile([P, T], F32)
            nc.vector.memset(ones32, 1.0)

            w_sb = {}
            wT_sb = {}   # transposed (and rstd-scaled for q/k/v) weights [c_in, c_out]
            bias_sb = {}
            for n in ("q", "k", "v", "o"):
                w_sb[n] = consts.tile([C, C], F32, tag=f"w_{n}", name=f"w_{n}")
                nc.sync.dma_start(out=w_sb[n], in_=w_e[n][:, :])
                wT_sb[n] = consts.tile([C, C], F32R, tag=f"wT_{n}", name=f"wT_{n}")
                bias_sb[n] = consts.tile([1, C], F32R, tag=f"b_{n}", name=f"b_{n}")
            # bo lives on partition 64 so the K=1 bias matmul can pair with the
            # rowsum row of oc_sb (operands must share a base partition). It is
            # placed there via a K=1 matmul (DMA can't satisfy the f32r
            # producer-rounding rule, DVE can't cross partitions).
            bo_hi = consts.tile([C + 1, C], F32R)
            bo_raw = consts.tile([1, C], F32)
            nc.sync.dma_start(out=bo_raw, in_=b_e["o"][:, :])

            nc.sync.dma_start(out=x_sb, in_=x_r)

            stats_sb = consts.tile([P, 3], F32)       # mean, var, mean^2 per partition
            moments = consts.tile([1, 4], F32)        # scratch for scalar math
            bvals = consts.tile([P, 2], F32)          # [mu, rstd] broadcast to all parts
            negmu = consts.tile([1, 1], F32)

            with tc.tile_pool(name="pre_ps", bufs=2, space="PSUM") as pps:
                # ---- GroupNorm stats: bn_stats per 512-chunk, then aggregate ----
                bnst = consts.tile([P, T * C // 512, 6], F32)
                x_flat = x_sb[:, :, :].rearrange("p t c -> p (t c)")
                for i in range(T * C // 512):
                    nc.vector.bn_stats(
                        out=bnst[:, i, :], in_=x_flat[:, bass.ts(i, 512)]
                    )
                nc.vector.bn_aggr(out=stats_sb[:, 0:2], in_=bnst)
                nc.vector.tensor_mul(
                    stats_sb[:, 2:3], stats_sb[:, 0:1], stats_sb[:, 0:1]
                )
                ssum_ps = pps.tile([1, 3], F32, tag="small")
                nc.tensor.matmul(ssum_ps, lhsT=ones_col, rhs=stats_sb)
                # moments: [E[mean_p], E[var_p], E[mean_p^2], _]
                nc.scalar.mul(moments[:, 0:3], ssum_ps, 1.0 / P)
                # var_total = E[var_p] + E[mean_p^2] - mu^2
                nc.vector.tensor_mul(moments[:, 3:4], moments[:, 0:1], moments[:, 0:1])
                nc.vector.tensor_sub(moments[:, 1:2], moments[:, 1:2], moments[:, 3:4])
                nc.vector.tensor_add(moments[:, 1:2], moments[:, 1:2], moments[:, 2:3])
                # rstd = exp(-0.5 * ln(var + eps)); Ln/Exp share one ACT table set
                eps_sb = consts.tile([1, 1], F32)
                nc.vector.memset(eps_sb, EPS)
                nc.scalar.activation(moments[:, 2:3], moments[:, 1:2], AF.Ln, bias=eps_sb)
                nc.scalar.activation(moments[:, 3:4], moments[:, 2:3], AF.Exp, scale=-0.5)
                nc.scalar.mul(negmu, moments[:, 0:1], -1.0)

                # place bo on partition 64, rounded to f32r
                bo_ps = pps.tile([C + 1, C], F32, tag="small", name="bo_ps")
                nc.tensor.matmul(
                    bo_ps[C : C + 1, :], lhsT=ones_col[0:1, 0:1], rhs=bo_raw
                )
                nc.vector.tensor_copy(bo_hi[C : C + 1, :], bo_ps[C : C + 1, :])

                # broadcast [mu, rstd] to all 128 partitions via K=1 matmul
                pair = consts.tile([1, 2], F32)
                nc.vector.tensor_copy(pair[:, 0:1], moments[:, 0:1])
                nc.vector.tensor_copy(pair[:, 1:2], moments[:, 3:4])
                bc_ps = pps.tile([P, 2], F32, tag="small")
                nc.tensor.matmul(bc_ps, lhsT=ones512_f[0:1, 0:P], rhs=pair)
                nc.vector.tensor_copy(bvals, bc_ps)

                # ---- weights: transpose, scale q/k/v by rstd, fold mu into bias ----
                for n in ("q", "k", "v", "o"):
                    wt_ps = pps.tile([C, C], F32, tag="small")
                    nc.tensor.transpose(wt_ps, w_sb[n], id128[0:C, 0:C])
                    if n == "o":
                        nc.vector.tensor_copy(wT_sb[n], wt_ps)
                        continue
                    nc.scalar.mul(wT_sb[n], wt_ps, bvals[0:C, 1:2])
                    cs_ps = pps.tile([1, C], F32, tag="small")
                    nc.tensor.matmul(cs_ps, lhsT=ones_col[0:C, :], rhs=wT_sb[n].bitcast(F32))
                    b_tmp = consts.tile([1, C], F32, tag=f"braw_{n}", name=f"braw_{n}")
                    nc.sync.dma_start(out=b_tmp, in_=b_e[n][:, :])
                    # bias' = b + (-mu) * colsum(rstd * W^T)
                    nc.vector.scalar_tensor_tensor(
                        out=bias_sb[n],
                        in0=cs_ps,
                        scalar=negmu,
                        in1=b_tmp,
                        op0=OP.mult,
                        op1=OP.add,
                    )

                # ---- x^T via PE transpose, 4 tiles per PSUM bank ----
                for gq in range(T // 4):
                    tp_ps = pps.tile([C, 4 * P], F32, tag="tp")
                    for i in range(4):
                        t = gq * 4 + i
                        nc.tensor.transpose(
                            tp_ps[:, bass.ts(i, P)], x_sb[:, t, :], id128
                        )
                    nc.vector.tensor_copy(xT_sb[:, bass.ts(gq, 4 * P)], tp_ps)

                # ---- q^T, k^T in fp8 DoubleRow packing: channel c = h*32 + kp
                for n, dst in (("q", qT_sb), ("k", kT_sb)):
                    for nb in range(NB):
                        for h in range(2):
                            qk_ps = pps.tile([C // 2, 512], F32, tag="qk", name="qk_ps")
                            nc.tensor.matmul(
                                qk_ps,
                                lhsT=wT_sb[n][:, bass.ts(h, C // 2)],
                                rhs=xT_sb[:, bass.ts(nb, 512)],
                                start=True,
                                stop=False,
                            )
                            nc.tensor.matmul(
                                qk_ps,
                                lhsT=bias_sb[n][:, bass.ts(h, C // 2)],
                                rhs=ones512,
                                start=False,
                                stop=True,
                            )
                            nc.vector.tensor_copy(dst[:, h, bass.ts(nb, 512)], qk_ps)

                # ---- v tiles [sj, c] + bias, ones column; padded to 80 for
                # DoubleRow's 16-byte Ko-step rule (cols 65..79 stay zero) ----
                nc.vector.memset(v_sb[:, :, :], 0.0)
                for gv in range(T // 8):
                    v_ps = pps.tile([P, 8, C], F32, tag="vps")
                    for i in range(8):
                        t = gv * 8 + i
                        nc.tensor.matmul(
                            v_ps[:, i, :],
                            lhsT=xT_sb[:, bass.ts(t, P)],
                            rhs=wT_sb["v"],
                            start=True,
                            stop=False,
                        )
                        nc.tensor.matmul(
                            v_ps[:, i, :],
                            lhsT=ones512[0:1, 0:P],
                            rhs=bias_sb["v"],
                            start=False,
                            stop=True,
                        )
                    nc.vector.tensor_copy(
                        v_sb[:, bass.ts(gv, 8), 0:C], v_ps
                    )
                nc.vector.tensor_copy(v_sb[:, :, C], ones32)

            # ---- main attention loop over si blocks of 512, software-pipelined:
            # block nb's score-groups interleave with block nb-1's A*V chunks so
            # the PE never stalls on the exp (ScalarE) stage. eT is
            # double-buffered so exp(nb) doesn't wait on A*V(nb-1) reads.
            with (
                tc.tile_pool(name="sc_ps", bufs=1, space="PSUM") as sc_pool,
                tc.tile_pool(name="ot_ps", bufs=1, space="PSUM") as ot_pool,
                tc.tile_pool(name="z_ps", bufs=1, space="PSUM") as z_pool,
                tc.tile_pool(name="tr_ps", bufs=2, space="PSUM") as tr_pool,
            ):
                eT_bufs = [eT_sb, eT_sb2]

                def emit_scores_group(nb, g):
                    si = bass.ts(nb, 512)
                    sc_ps = sc_pool.tile([P, 4, 512], F32, tag="sc", name="sc_ps")
                    for i in range(4):
                        sj = g * 4 + i
                        nc.tensor.matmul(
                            sc_ps[:, i, :],
                            lhsT=kT_sb[:, :, bass.ts(sj, P)],
                            rhs=qT_sb[:, :, si],
                            start=True,
                            stop=True,
                            perf_mode=DR,
                        )
                    nc.scalar.activation(
                        out=eT_bufs[nb % 2][:, bass.ts(g, 4), :],
                        in_=sc_ps,
                        func=AF.Exp,
                        scale=float(C) ** -0.5,
                    )

                def emit_av_chunk(nb, g, ot_ps):
                    eT = eT_bufs[nb % 2]
                    for i in range(2):
                        t2 = g * 2 + i
                        nc.tensor.matmul(
                            ot_ps,
                            lhsT=v_sb[:, 2 * t2 : 2 * t2 + 2, :],
                            rhs=eT[:, 2 * t2 : 2 * t2 + 2, :],
                            start=(t2 == 0),
                            stop=(t2 == T // 2 - 1),
                            perf_mode=DR,
                        )

                def emit_tail(nb, ot_ps):
                    oc_sb = work.tile([C + 1, 512], F32R, tag="oc", name="oc_sb")
                    nc.vector.tensor_copy(oc_sb, ot_ps[0 : C + 1, :])
                    # z^T = Wo @ o^T + bo x rowsum (divide happens post-transpose)
                    z_ps = z_pool.tile([C, 512], F32, tag="z", name="z_ps")
                    nc.tensor.matmul(
                        z_ps,
                        lhsT=wT_sb["o"],
                        rhs=oc_sb[0:C, :],
                        start=True,
                        stop=False,
                    )
                    nc.tensor.matmul(
                        z_ps,
                        lhsT=bo_hi[C : C + 1, :],
                        rhs=oc_sb[C : C + 1, :],
                        start=False,
                        stop=True,
                    )
                    zc_sb = work.tile([C + 1, 512], F32, tag="zc", name="zc_sb")
                    nc.vector.tensor_copy(zc_sb[0:C, :], z_ps)
                    nc.vector.tensor_copy(zc_sb[C : C + 1, :], oc_sb[C : C + 1, :])
                    # transpose back to [si, c], divide by rowsum, add residual
                    out_sb = work.tile([P, 4, C], F32, tag="outt", name="out_sb")
                    for q4 in range(4):
                        tr_ps = tr_pool.tile([P, C + 1], F32, tag="tr", name="tr_ps")
                        nc.tensor.transpose(
                            tr_ps, zc_sb[:, bass.ts(q4, P)], id128[0 : C + 1, 0 : C + 1]
                        )
                        rec_sb = work.tile([P, 1], F32, tag="rec", name="rec_sb")
                        nc.vector.reciprocal(rec_sb, tr_ps[:, C : C + 1])
                        nc.vector.scalar_tensor_tensor(
                            out=out_sb[:, q4, :],
                            in0=tr_ps[:, 0:C],
                            scalar=rec_sb,
                            in1=x_sb[:, nb * 4 + q4, :],
                            op0=OP.mult,
                            op1=OP.add,
                        )
                    nc.sync.dma_start(out=out_r[nb], in_=out_sb)

                ot_live = {}
                for nb in range(NB):
                    for g in range(T // 4):
                        emit_scores_group(nb, g)
                        if nb >= 1:
                            if g == 0:
                                ot_live[nb - 1] = ot_pool.tile(
                                    [80, 512], F32, tag="ot", name="ot_ps"
                                )
                            emit_av_chunk(nb - 1, g, ot_live[nb - 1])
                    if nb >= 1:
                        emit_tail(nb - 1, ot_live.pop(nb - 1))
                ot_live[NB - 1] = ot_pool.tile([80, 512], F32, tag="ot", name="ot_ps")
                for g in range(T // 4):
                    emit_av_chunk(NB - 1, g, ot_live[NB - 1])
                emit_tail(NB - 1, ot_live.pop(NB - 1))

    nc.finalize()
    return nc


def _get_nc():
    global _CACHED_NC
    if _CACHED_NC is None:
        _CACHED_NC = build_nc()
    return _CACHED_NC


def kernel(x, temb, Wq, bq, Wk, bk, Wv, bv, Wo, bo, **_unused):
    global LAST_RESULTS
    nc = _get_nc()
    x = np.ascontiguousarray(np.asarray(x, dtype=np.float32))
    shared = {
        "Wq": np.ascontiguousarray(Wq, dtype=np.float32),
        "Wk": np.ascontiguousarray(Wk, dtype=np.float32),
        "Wv": np.ascontiguousarray(Wv, dtype=np.float32),
        "Wo": np.ascontiguousarray(Wo, dtype=np.float32),
        "bq": np.asarray(bq, dtype=np.float32).reshape(1, C),
        "bk": np.asarray(bk, dtype=np.float32).reshape(1, C),
        "bv": np.asarray(bv, dtype=np.float32).reshape(1, C),
        "bo": np.asarray(bo, dtype=np.float32).reshape(1, C),
    }
    in_maps = [{"x": x[i].reshape(S, C), **shared} for i in range(B)]
    res = run_bass_kernel_spmd(nc, in_maps, core_ids=list(range(B)))
    LAST_RESULTS = res
    out = np.stack([res.results[i]["out"].reshape(H, W, C) for i in range(B)])
    return out.astype(np.float32)
